# revision 3
# baseline (speedup 1.0000x reference)
"""Trainium2 Bass kernel for nn_AttentionMechanism (Bahdanau-style attention).

Reference computation (per batch b):
    dec_proj = decoder_hidden @ W_dec + b_attn            # (H,)
    enc_proj = encoder_outputs[b] @ W_enc                 # (S, H)
    energy   = tanh(dec_proj + enc_proj)                  # (S, H)
    scores   = energy @ v                                 # (S,)
    attn     = softmax(scores)                            # (S,)
    context  = attn @ encoder_outputs[b]                  # (H,)

Sharding: data-parallel over batch, 8 batches per core on 8 cores.

Per-core dataflow (all matmuls in bf16, accumulation fp32 in PSUM):
  - enc[b] is loaded HBM->SBUF with an inline fp32->bf16 cast (SWDGE DMA),
    laid out [128 s-part, 16 s-tile, 512 h].
  - One big SBUF->SBUF DMA xbar transpose turns that into encT
    (h on partitions), which feeds the main matmul
    energyT[k, s] = sum_h W_enc[h, k] * enc[s, h] (+ bias via ACT).
  - tanh+bias fused on the scalar engine (bias = dec_proj+b_attn is
    per-partition in this layout), output bf16.
  - scores via PE (v as stationary), softmax without max-subtraction
    (|scores| <= sum|v| ~ 22.6, safe in fp32), exp+sum fused on ACT.
  - context via PE with the (reshaped) weights as stationary.
"""

import numpy as np

H = 512
S = 2048
B = 64
N_CORES = 8
BC = B // N_CORES  # batches per core
P = 128
JT = S // P   # 16 s-tiles
KT = H // P   # 4 k-tiles
HT = H // P   # 4 h-tiles

_COMPILED = {}


def _build():
    import concourse.bacc as bacc
    import concourse.bass as bass
    import concourse.mybir as mybir
    import concourse.tile as tile

    f32 = mybir.dt.float32
    bf16 = mybir.dt.bfloat16
    AF = mybir.ActivationFunctionType
    ALU = mybir.AluOpType
    AX = mybir.AxisListType

    nc = bacc.Bacc("TRN2", target_bir_lowering=False, debug=False)

    enc_d = nc.dram_tensor("enc", [BC, S, H], f32, kind="ExternalInput").ap()
    decT_d = nc.dram_tensor("decT", [H, BC], f32, kind="ExternalInput").ap()
    w_d = nc.dram_tensor("w_attn", [2 * H, H], f32, kind="ExternalInput").ap()
    ba_d = nc.dram_tensor("b_attn", [1, H], f32, kind="ExternalInput").ap()
    v_d = nc.dram_tensor("v_vec", [H, 1], f32, kind="ExternalInput").ap()
    ctx_d = nc.dram_tensor("ctx_out", [BC, H], f32, kind="ExternalOutput").ap()
    attn_d = nc.dram_tensor("attn_out", [BC, S], f32, kind="ExternalOutput").ap()

    with tile.TileContext(nc) as tc:
        with tc.tile_pool(name="const", bufs=1) as cpool:
            # W_attn as [128, 8 row-tiles, 512]; tiles 0-3 = W_dec, 4-7 = W_enc
            w_sb = cpool.tile([P, 8, H], f32)
            nc.sync.dma_start(w_sb[:], w_d.rearrange("(t p) k -> p t k", p=P))
            wenc_bf = cpool.tile([P, HT, H], bf16)
            nc.vector.tensor_copy(wenc_bf[:], w_sb[:, 4:8, :])

            decT_sb = cpool.tile([P, HT, BC], f32)
            nc.sync.dma_start(decT_sb[:], decT_d.rearrange("(t p) b -> p t b", p=P))
            ba_sb = cpool.tile([1, H], f32)
            nc.sync.dma_start(ba_sb[:], ba_d)
            v_sb = cpool.tile([P, KT, 1], f32)
            nc.sync.dma_start(v_sb[:], v_d.rearrange("(t p) a -> p t a", p=P))
            v_bf = cpool.tile([P, KT, 1], bf16)
            nc.vector.tensor_copy(v_bf[:], v_sb[:])
            ones8 = cpool.tile([1, BC], f32)
            nc.vector.memset(ones8[:], 1.0)

            # biasT[k, kt, b] = dec_proj(b)[kt*128+k] + b_attn[kt*128+k]
            biasT = cpool.tile([P, KT, BC], f32)
            with tc.tile_pool(name="setup_ps", bufs=1, space="PSUM") as spool:
                dec_ps = spool.tile([P, KT, BC], f32)
                for kt in range(KT):
                    for t in range(HT):
                        nc.tensor.matmul(
                            dec_ps[:, kt, :],
                            w_sb[:, t, kt * P : (kt + 1) * P],
                            decT_sb[:, t, :],
                            start=(t == 0),
                            stop=False,
                        )
                    nc.tensor.matmul(
                        dec_ps[:, kt, :],
                        ba_sb[:, kt * P : (kt + 1) * P],
                        ones8[:],
                        start=False,
                        stop=True,
                    )
                nc.scalar.copy(biasT[:], dec_ps[:])

            with (
                tc.tile_pool(name="encp", bufs=2) as encpool,
                tc.tile_pool(name="encTp", bufs=2) as encTpool,
                tc.tile_pool(name="energyp", bufs=8) as epool,
                tc.tile_pool(name="smallp", bufs=2) as sm,
                tc.tile_pool(name="eps_ps", bufs=2, space="PSUM") as eps_pool,
                tc.tile_pool(name="sc_ps", bufs=2, space="PSUM") as sc_pool,
                tc.tile_pool(name="ctx_ps", bufs=2, space="PSUM") as ctx_pool,
                tc.tile_pool(name="dramp", bufs=2, space="DRAM") as dpool,
            ):
                for b in range(BC):
                    # --- load enc[b] with inline fp32->bf16 cast ---
                    enc_sb = encpool.tile([P, JT, H], bf16, tag="enc_sb")
                    src = enc_d[b].rearrange("(j p) h -> p j h", p=P)
                    half = JT // 2
                    nc.gpsimd.dma_start(enc_sb[:, :half, :], src[:, :half, :])
                    nc.gpsimd.dma_start(enc_sb[:, half:, :], src[:, half:, :])

                    # --- transpose to h-on-partitions via DMA xbar ---
                    # encT[h%128, p*64 + j*4 + h//128] = enc[j*128+p, h]
                    encT = encTpool.tile([P, P * JT * HT], bf16, tag="encT")
                    nc.sync.dma_start(
                        encT[:],
                        enc_sb[:].rearrange("p j h -> p (j h)"),
                        transpose=True,
                    )
                    encT_v = encT[:].rearrange(
                        "a (p j t) -> a t j p", p=P, j=JT, t=HT
                    )

                    # --- energyT[k, s] = tanh(bias[k] + sum_h W_enc[h,k] enc[s,h]) ---
                    energies = []
                    for kt in range(KT):
                        energy = epool.tile([P, S], bf16, tag="energy")
                        energies.append(energy)
                        for sh in range(2):
                            e_ps = eps_pool.tile([P, S // 2], f32, tag="eps")
                            for sc in range(2):
                                j0 = sh * 8 + sc * 4
                                for ht in range(HT):
                                    nc.tensor.matmul(
                                        e_ps[:, sc * 512 : (sc + 1) * 512],
                                        wenc_bf[:, ht, kt * P : (kt + 1) * P],
                                        encT_v[:, ht, j0 : j0 + 4, :],
                                        start=(ht == 0),
                                        stop=(ht == HT - 1),
                                    )
                            nc.scalar.activation(
                                energy[:, sh * 1024 : (sh + 1) * 1024],
                                e_ps[:],
                                AF.Tanh,
                                bias=biasT[:, kt, b : b + 1],
                            )

                    # --- scores + exp + partial sums ---
                    exp_sb = sm.tile([1, S], f32, tag="exp_sb")
                    partials = sm.tile([1, 4], f32, tag="partials")
                    for sc4 in range(4):
                        sc_ps = sc_pool.tile([1, 512], f32, tag="sc")
                        for kt in range(KT):
                            nc.tensor.matmul(
                                sc_ps[:],
                                v_bf[:, kt, :],
                                energies[kt][:, sc4 * 512 : (sc4 + 1) * 512],
                                start=(kt == 0),
                                stop=(kt == KT - 1),
                            )
                        nc.scalar.activation(
                            exp_sb[:, sc4 * 512 : (sc4 + 1) * 512],
                            sc_ps[:],
                            AF.Exp,
                            accum_out=partials[:, sc4 : sc4 + 1],
                        )

                    # --- softmax normalization ---
                    total = sm.tile([1, 1], f32, tag="total")
                    nc.vector.tensor_reduce(total[:], partials[:], axis=AX.X, op=ALU.add)
                    recip = sm.tile([1, 1], f32, tag="recip")
                    nc.vector.reciprocal(recip[:], total[:])
                    w_f32 = sm.tile([1, S], f32, tag="w_f32")
                    nc.vector.tensor_scalar_mul(w_f32[:], exp_sb[:], recip[:])
                    nc.sync.dma_start(attn_d[b : b + 1, :], w_f32[:])

                    # --- reshape weights to s-partitioned [128, 16] (bf16) ---
                    # SBUF [1,S] -> [128,16] crosses partitions, so bounce via a
                    # DRAM scratch tile; the reload casts fp32->bf16 inline.
                    wscr = dpool.tile([1, S], f32, tag="wscr")
                    nc.sync.dma_start(wscr[:], w_f32[:])
                    w_sp = sm.tile([P, JT], bf16, tag="w_sp")
                    nc.gpsimd.dma_start(
                        w_sp[:], wscr[:].rearrange("a (j p) -> p (a j)", p=P)
                    )

                    # --- context[h] = sum_s attn[s] * enc[s, h] ---
                    ctx_ps = ctx_pool.tile([1, H], f32, tag="ctx")
                    for j in range(JT):
                        nc.tensor.matmul(
                            ctx_ps[:],
                            w_sp[:, j : j + 1],
                            enc_sb[:, j, :],
                            start=(j == 0),
                            stop=(j == JT - 1),
                        )
                    ctx_sb = sm.tile([1, H], f32, tag="ctx_sb")
                    nc.vector.tensor_copy(ctx_sb[:], ctx_ps[:])
                    nc.sync.dma_start(ctx_d[b : b + 1, :], ctx_sb[:])

    nc.compile()
    return nc


def _get_compiled():
    if "nc" not in _COMPILED:
        _COMPILED["nc"] = _build()
    return _COMPILED["nc"]


def _make_in_maps(decoder_hidden, encoder_outputs, W_attn, b_attn, v):
    W = np.ascontiguousarray(W_attn, dtype=np.float32)
    ba = np.ascontiguousarray(b_attn, dtype=np.float32).reshape(1, H)
    vv = np.ascontiguousarray(v, dtype=np.float32).reshape(H, 1)
    in_maps = []
    for c in range(N_CORES):
        sl = slice(c * BC, (c + 1) * BC)
        in_maps.append(
            {
                "enc": np.ascontiguousarray(encoder_outputs[sl], dtype=np.float32),
                "decT": np.ascontiguousarray(
                    decoder_hidden[sl].T, dtype=np.float32
                ),
                "w_attn": W,
                "b_attn": ba,
                "v_vec": vv,
            }
        )
    return in_maps


def run(decoder_hidden, encoder_outputs, W_attn, b_attn, v, **spmd_kwargs):
    from concourse.bass_utils import run_bass_kernel_spmd

    nc = _get_compiled()
    in_maps = _make_in_maps(decoder_hidden, encoder_outputs, W_attn, b_attn, v)
    res = run_bass_kernel_spmd(nc, in_maps, list(range(N_CORES)), **spmd_kwargs)
    context = np.concatenate(
        [np.asarray(res.results[c]["ctx_out"], dtype=np.float32) for c in range(N_CORES)], axis=0
    )
    attn = np.concatenate(
        [np.asarray(res.results[c]["attn_out"], dtype=np.float32) for c in range(N_CORES)], axis=0
    )
    return (context, attn), res


def kernel(decoder_hidden, encoder_outputs, W_attn, b_attn, v):
    (context, attn), _ = run(decoder_hidden, encoder_outputs, W_attn, b_attn, v)
    return context, attn


if __name__ == "__main__":
    rng = np.random.default_rng(0)
    inputs = {
        "decoder_hidden": rng.standard_normal((B, H), dtype=np.float32),
        "encoder_outputs": rng.standard_normal((B, S, H), dtype=np.float32),
        "W_attn": (rng.random((2 * H, H), dtype=np.float32) - 0.5) * 2 / np.sqrt(2 * H),
        "b_attn": (rng.random(H, dtype=np.float32) - 0.5) * 2 / np.sqrt(2 * H),
        "v": (rng.random(H, dtype=np.float32) - 0.5) * 2 / np.sqrt(H),
    }
    ctx, attn = kernel(**inputs)
    print("ctx", ctx.shape, ctx.dtype, "attn", attn.shape, attn.dtype)
